# revision 26
# baseline (speedup 1.0000x reference)
"""CycleFC per-channel W-shift kernel for 8 TRN2 NeuronCores.

Problem: x [32, 256, 64, 64] f32. out[b,c,h,w] = x[b,c,h,w-s] when
0 <= w-s < 64 else 0, with s = BASE[c % 8], BASE = [-2,-1,0,1,2,1,0,-1].

Sharding: data-parallel on batch, 4 batches per core, no communication.

Per-core scheme (pure data movement, HBM-roofline bound; submitted
variant "v3" = _build_slots_h2, 16 pipeline units of 1 MiB):
  view x as [4, 32, 8, H*W]  (b, c_hi, c%8, flat spatial)
  for each channel class p (shift s) and H-half:
    - DMA-load the flat block shifted by s elements into an SBUF tile
      [128 part = (b, c_hi), 2048] -> ~8 KiB contiguous runs (HWDGE/sync)
    - DVE-memset the per-row edge columns (w < s or w >= W+s) to zero
    - DMA-store the tile back fully aligned (HWDGE/scalar)
  Loads and stores ride separate HWDGE rings; per-unit semaphores chain
  load -> memset -> store; units pipeline freely against each other.
  Measured 92-124 us/pass across sessions (~94 us HBM roofline).

Other builder variants in this file (v1/v2/pair/split2/aff/ph/d2d/...)
are the experiments that selected v3; kernel() uses v3 only.
"""

import numpy as np

import concourse.bass as bass
import concourse.mybir as mybir
from concourse.bass_utils import run_bass_kernel_spmd

B, C, H, W = 32, 256, 64, 64
HW = H * W  # 4096
N_CORES = 8
B_SH = B // N_CORES  # 4
C_HI = C // 8  # 32
BASE = [-2, -1, 0, 1, 2, 1, 0, -1]  # shift per (c % 8)

_cached_nc = None


def _build(reps: int = 1, variant: str = "v1") -> bass.Bass:
    """variant:
    v1      - one load/memset/store unit per channel class (8 units)
    pair    - classes with equal shift share one unit (5 units)
    split2  - each class split into 2 DMAs along batch (8 units, 2 DMAs each)
    noshift - v1 with all shifts forced 0 (WRONG output; alignment probe)
    """
    from contextlib import ExitStack

    nc = bass.Bass()
    x = nc.declare_dram_parameter(
        "x", [B_SH, C_HI, 8, HW], mybir.dt.float32, isOutput=False
    )
    out = nc.declare_dram_parameter(
        "out", [B_SH, C_HI, 8, HW], mybir.dt.float32, isOutput=True
    )

    if variant == "aff":
        return _build_aff(nc, x, out, reps)
    if variant.startswith("v2"):
        nslots = int(variant[2:]) if len(variant) > 2 else 12
        return _build_slots(nc, x, out, reps, nslots)
    if variant == "ph":
        return _build_phased(nc, x, out, reps)
    if variant in ("ldonly", "d2draw", "d2d"):
        return _build_d2d(nc, x, out, reps, variant)
    if variant == "ldwide":
        return _build_ldwide(nc, x, out, reps)
    if variant.startswith("v3"):
        rest = variant[2:]
        gp_store = rest.startswith("g")
        if gp_store:
            rest = rest[1:]
        nslots = int(rest) if rest else 20
        return _build_slots_h2(nc, x, out, reps, nslots, split=2, gp_store=gp_store)
    if variant.startswith("v4"):
        nslots = int(variant[2:]) if len(variant) > 2 else 32
        return _build_slots_h2(nc, x, out, reps, nslots, split=4)

    # units: (name, class-tuple, shift)
    if variant == "pair":
        units = [
            ((0,), -2),
            ((1, 7), -1),
            ((2, 6), 0),
            ((3, 5), 1),
            ((4,), 2),
        ]
    elif variant == "noshift":
        units = [((p,), 0) for p in range(8)]
    else:  # v1, split2
        units = [((p,), BASE[p]) for p in range(8)]

    n_dma = 2 if variant == "split2" else 1  # DMAs per load/store unit
    U = len(units)

    def src_ap(ps, lo, hi):
        """x[:, :, ps, lo:hi] as one AP (ps is a stride-regular tuple)."""
        if len(ps) == 1:
            return x[:, :, ps[0], lo:hi]
        step = ps[1] - ps[0]
        return x[:, :, ps[0] : ps[1] + 1 : step, lo:hi]

    def dst_ap(ps):
        if len(ps) == 1:
            return out[:, :, ps[0], :]
        step = ps[1] - ps[0]
        return out[:, :, ps[0] : ps[1] + 1 : step, :]

    with ExitStack() as stack:
        tiles = [
            stack.enter_context(
                nc.sbuf_tensor(f"tile{u}", [128, len(ps) * HW], mybir.dt.float32)
            )
            for u, (ps, _) in enumerate(units)
        ]
        ld = [stack.enter_context(nc.semaphore(f"ld{u}")) for u in range(U)]
        ve = [stack.enter_context(nc.semaphore(f"ve{u}")) for u in range(U)]
        st = [stack.enter_context(nc.semaphore(f"st{u}")) for u in range(U)]
        blk = stack.enter_context(nc.Block())

        @blk.sync
        def _(sync):
            for r in range(reps):
                for u, (ps, s) in enumerate(units):
                    if r > 0:
                        sync.wait_ge(st[u], 16 * n_dma * r)  # WAR: prev store done
                    lo, hi = max(0, -s), HW + min(0, -s)
                    tl, th = max(0, s), HW + min(0, s)
                    t3 = tiles[u][:].rearrange("p (q f) -> p q f", f=HW)
                    if n_dma == 1:
                        sync.dma_start(
                            out=t3[:, :, tl:th], in_=src_ap(ps, lo, hi)
                        ).then_inc(ld[u], 16)
                    else:
                        half = 64  # partitions per half (= 2 of 4 batches)
                        sync.dma_start(
                            out=t3[0:half, :, tl:th],
                            in_=src_ap(ps, lo, hi)[0 : B_SH // 2],
                        ).then_inc(ld[u], 16)
                        sync.dma_start(
                            out=t3[half:128, :, tl:th],
                            in_=src_ap(ps, lo, hi)[B_SH // 2 : B_SH],
                        ).then_inc(ld[u], 16)

        @blk.vector
        def _(vector):
            for r in range(reps):
                for u, (ps, s) in enumerate(units):
                    if s == 0:
                        continue
                    vector.wait_ge(ld[u], 16 * n_dma * (r + 1))
                    rr = tiles[u][:].rearrange("p (q h w) -> p q h w", h=H, w=W)
                    if s > 0:
                        vector.memset(rr[:, :, :, 0:s], 0.0).then_inc(ve[u], 1)
                    else:
                        vector.memset(rr[:, :, :, W + s : W], 0.0).then_inc(ve[u], 1)

        @blk.scalar
        def _(scalar):
            for r in range(reps):
                for u, (ps, s) in enumerate(units):
                    if s == 0:
                        scalar.wait_ge(ld[u], 16 * n_dma * (r + 1))
                    else:
                        scalar.wait_ge(ve[u], r + 1)
                    if n_dma == 1:
                        scalar.dma_start(out=dst_ap(ps), in_=tiles[u][:]).then_inc(
                            st[u], 16
                        )
                    else:
                        scalar.dma_start(
                            out=dst_ap(ps)[0 : B_SH // 2], in_=tiles[u][0:64]
                        ).then_inc(st[u], 16)
                        scalar.dma_start(
                            out=dst_ap(ps)[B_SH // 2 : B_SH], in_=tiles[u][64:128]
                        ).then_inc(st[u], 16)
            for u in range(U):
                scalar.wait_ge(st[u], 16 * n_dma * reps)

    return nc


def _build_slots_h2(
    nc: bass.Bass, x, out, reps: int, nslots: int, split: int = 2, gp_store: bool = False
) -> bass.Bass:
    """Like _build_slots but each class is split into `split` H-chunks:
    8*split units per pass. Finer pipeline granularity shortens the
    single-pass ramp (first store starts after ~1 MiB instead of ~2 MiB)
    and the tail.

    Unit (p, hh) covers out-flat positions [hh*HW2, (hh+1)*HW2) of class p,
    where HW2 = HW/split (a whole number of H rows, so the per-row edge
    memset pattern is unchanged). The load reads x-flat [hh*HW2 - s, ...)
    clipped to [0, HW). gp_store issues stores on the gpsimd (SWDGE) queue
    instead of the scalar HWDGE ring.
    """
    from contextlib import ExitStack

    HW2 = HW // split
    UPP = 8 * split  # units per pass
    G = reps * UPP
    nslots = min(nslots, G)

    with ExitStack() as stack:
        tiles = [
            stack.enter_context(
                nc.sbuf_tensor(f"slot{k}", [128, HW2], mybir.dt.float32)
            )
            for k in range(nslots)
        ]
        ld = [stack.enter_context(nc.semaphore(f"ld{k}")) for k in range(nslots)]
        ve = [stack.enter_context(nc.semaphore(f"ve{k}")) for k in range(nslots)]
        st = [stack.enter_context(nc.semaphore(f"st{k}")) for k in range(nslots)]
        blk = stack.enter_context(nc.Block())

        def unit(g):
            j = g % UPP
            p, hh = j % 8, j // 8
            return p, hh, g % nslots, g // nslots

        @blk.sync
        def _(sync):
            for g in range(G):
                p, hh, k, u = unit(g)
                s = BASE[p]
                # tile[j'] = x[hh*HW2 + j' - s] for valid; src range in x-flat:
                lo = max(0, hh * HW2 - s)
                hi = min(HW, (hh + 1) * HW2 - s)
                tl = lo - (hh * HW2 - s)  # dst offset within tile
                if u > 0:
                    sync.wait_ge(st[k], 16 * u)
                sync.dma_start(
                    out=tiles[k][:, tl : tl + (hi - lo)], in_=x[:, :, p, lo:hi]
                ).then_inc(ld[k], 16)

        @blk.vector
        def _(vector):
            for g in range(G):
                p, hh, k, u = unit(g)
                s = BASE[p]
                if s == 0:
                    continue
                vector.wait_ge(ld[k], 16 * (u + 1))
                rr = tiles[k][:].rearrange("p (h w) -> p h w", w=W)
                if s > 0:
                    vector.memset(rr[:, :, 0:s], 0.0).then_inc(ve[k], 1)
                else:
                    vector.memset(rr[:, :, W + s : W], 0.0).then_inc(ve[k], 1)

        def store_prog(eng):
            ve_done = [0] * nslots
            st_done = [0] * nslots
            for g in range(G):
                p, hh, k, u = unit(g)
                s = BASE[p]
                if s == 0:
                    eng.wait_ge(ld[k], 16 * (u + 1))
                else:
                    ve_done[k] += 1
                    eng.wait_ge(ve[k], ve_done[k])
                eng.dma_start(
                    out=out[:, :, p, hh * HW2 : (hh + 1) * HW2], in_=tiles[k][:]
                ).then_inc(st[k], 16)
                st_done[k] += 1
            for k in range(nslots):
                eng.wait_ge(st[k], 16 * st_done[k])

        if gp_store:

            @blk.gpsimd
            def _(gp):
                store_prog(gp)

        else:

            @blk.scalar
            def _(scalar):
                store_prog(scalar)

    return nc


def _build_ldwide(nc: bass.Bass, x, out, reps: int) -> bass.Bass:
    """Load-only control with 2 classes per tile: 4 DMAs/rep of [128, 2*HW]
    with 32 KiB contiguous runs -> half the descriptors of ldonly. WRONG
    output; isolates whether HWDGE descriptor generation rate binds.
    """
    from contextlib import ExitStack

    with ExitStack() as stack:
        tiles = [
            stack.enter_context(
                nc.sbuf_tensor(f"tile{q}", [128, 2 * HW], mybir.dt.float32)
            )
            for q in range(4)
        ]
        ld = [stack.enter_context(nc.semaphore(f"ld{q}")) for q in range(4)]
        blk = stack.enter_context(nc.Block())

        @blk.sync
        def _(sync):
            for r in range(reps):
                for q in range(4):
                    # classes 2q, 2q+1 are adjacent: x[:, :, 2q:2q+2, :] is
                    # one 32 KiB contiguous run per (b, c_hi)
                    sync.dma_start(
                        out=tiles[q][:], in_=x[:, :, 2 * q : 2 * q + 2, :]
                    ).then_inc(ld[q], 16)
            for q in range(4):
                sync.wait_ge(ld[q], 16 * reps)

    return nc


def _build_d2d(nc: bass.Bass, x, out, reps: int, kind: str) -> bass.Bass:
    """DRAM->DRAM family.

    ldonly: HBM->SBUF loads only (WRONG output; pure-read rate control)
    d2draw: 8 shifted DRAM->DRAM block copies, no edge fix (WRONG output)
    d2d:    d2draw + per-row edge zeros DMA'd from a zeroed SBUF tile
    """
    from contextlib import ExitStack

    with ExitStack() as stack:
        if kind == "ldonly":
            tiles = [
                stack.enter_context(
                    nc.sbuf_tensor(f"tile{p}", [128, HW], mybir.dt.float32)
                )
                for p in range(8)
            ]
            ld = [stack.enter_context(nc.semaphore(f"ld{p}")) for p in range(8)]
            blk = stack.enter_context(nc.Block())

            @blk.sync
            def _(sync):
                for r in range(reps):
                    for p in range(8):
                        sync.dma_start(out=tiles[p][:], in_=x[:, :, p, :]).then_inc(
                            ld[p], 16
                        )
                for p in range(8):
                    sync.wait_ge(ld[p], 16 * reps)

            return nc

        zt = stack.enter_context(nc.sbuf_tensor("zt", [128, 128], mybir.dt.float32))
        st = [stack.enter_context(nc.semaphore(f"st{p}")) for p in range(8)]
        ez = [stack.enter_context(nc.semaphore(f"ez{p}")) for p in range(8)]
        vz = stack.enter_context(nc.semaphore("vz"))
        blk = stack.enter_context(nc.Block())

        @blk.vector
        def _(vector):
            if kind == "d2d":
                vector.memset(zt[:], 0.0).then_inc(vz, 1)

        @blk.sync
        def _(sync):
            for r in range(reps):
                for p in range(8):
                    s = BASE[p]
                    lo, hi = max(0, -s), HW + min(0, -s)
                    tl, th = max(0, s), HW + min(0, s)
                    sync.dma_start(
                        out=out[:, :, p, tl:th], in_=x[:, :, p, lo:hi]
                    ).then_inc(st[p], 16)
            for p in range(8):
                sync.wait_ge(st[p], 16 * reps)

        if kind == "d2d":

            @blk.gpsimd
            def _(gp):
                gp.wait_ge(vz, 1)
                for r in range(reps):
                    for p in range(8):
                        s = BASE[p]
                        if s == 0:
                            continue
                        gp.wait_ge(st[p], 16 * (r + 1))
                        o4 = out[:, :, p, :].rearrange("b c (h w) -> b c h w", w=W)
                        if s > 0:
                            dst = o4[:, :, :, 0:s]
                        else:
                            dst = o4[:, :, :, W + s : W]
                        with nc.allow_non_contiguous_dma(
                            reason="per-row edge zeros: |s| elems per row"
                        ):
                            gp.dma_start(out=dst, in_=zt[:, 0 : H * abs(s)]).then_inc(
                                ez[p], 16
                            )
                nz = sum(1 for p in range(8) if BASE[p] != 0)
                for p in range(8):
                    if BASE[p] != 0:
                        gp.wait_ge(ez[p], 16 * reps)

    return nc


def _build_phased(nc: bass.Bass, x, out, reps: int) -> bass.Bass:
    """v1 structure, but the store phase is gated on ALL loads/memsets of the
    pass: HBM sees a pure-read phase then a pure-write phase, avoiding
    read/write bus-turnaround mixing penalties. Memsets overlap the tail of
    the load phase. HBM is the only binding resource, so phasing loses no
    overlap; it only removes R/W interleaving.
    """
    from contextlib import ExitStack

    with ExitStack() as stack:
        tiles = [
            stack.enter_context(nc.sbuf_tensor(f"tile{p}", [128, HW], mybir.dt.float32))
            for p in range(8)
        ]
        ld = [stack.enter_context(nc.semaphore(f"ld{p}")) for p in range(8)]
        ve = [stack.enter_context(nc.semaphore(f"ve{p}")) for p in range(8)]
        st = [stack.enter_context(nc.semaphore(f"st{p}")) for p in range(8)]
        blk = stack.enter_context(nc.Block())

        @blk.sync
        def _(sync):
            for r in range(reps):
                if r > 0:
                    for p in range(8):
                        sync.wait_ge(st[p], 16 * r)  # write phase r-1 drained
                for p in range(8):
                    s = BASE[p]
                    if s >= 0:
                        sync.dma_start(
                            out=tiles[p][:, s:HW], in_=x[:, :, p, 0 : HW - s]
                        ).then_inc(ld[p], 16)
                    else:
                        sync.dma_start(
                            out=tiles[p][:, 0 : HW + s], in_=x[:, :, p, -s:HW]
                        ).then_inc(ld[p], 16)

        @blk.vector
        def _(vector):
            for r in range(reps):
                for p in range(8):
                    s = BASE[p]
                    if s == 0:
                        continue
                    vector.wait_ge(ld[p], 16 * (r + 1))
                    rr = tiles[p][:].rearrange("p (h w) -> p h w", w=W)
                    if s > 0:
                        vector.memset(rr[:, :, 0:s], 0.0).then_inc(ve[p], 1)
                    else:
                        vector.memset(rr[:, :, W + s : W], 0.0).then_inc(ve[p], 1)

        @blk.scalar
        def _(scalar):
            for r in range(reps):
                # gate: whole read phase (incl. memsets) done before any store
                for p in range(8):
                    s = BASE[p]
                    if s == 0:
                        scalar.wait_ge(ld[p], 16 * (r + 1))
                    else:
                        scalar.wait_ge(ve[p], r + 1)
                for p in range(8):
                    scalar.dma_start(out=out[:, :, p, :], in_=tiles[p][:]).then_inc(
                        st[p], 16
                    )
            for p in range(8):
                scalar.wait_ge(st[p], 16 * reps)

    return nc


def _build_slots(nc: bass.Bass, x, out, reps: int, nslots: int) -> bass.Bass:
    """v1 structure with a rotating pool of tile buffers so that, across the
    benchmark rep loop, unit g's load only waits for the store of unit
    g-nslots — a deep pipeline window that removes the per-unit
    load->store->load serialization. With reps=1 (the graded single pass)
    only 8 slots are touched and this is identical to v1.
    """
    from contextlib import ExitStack

    G = reps * 8
    nslots = min(nslots, G)

    with ExitStack() as stack:
        tiles = [
            stack.enter_context(nc.sbuf_tensor(f"slot{k}", [128, HW], mybir.dt.float32))
            for k in range(nslots)
        ]
        ld = [stack.enter_context(nc.semaphore(f"ld{k}")) for k in range(nslots)]
        ve = [stack.enter_context(nc.semaphore(f"ve{k}")) for k in range(nslots)]
        st = [stack.enter_context(nc.semaphore(f"st{k}")) for k in range(nslots)]
        blk = stack.enter_context(nc.Block())

        @blk.sync
        def _(sync):
            for g in range(G):
                p = g % 8
                k = g % nslots
                u = g // nslots
                s = BASE[p]
                if u > 0:
                    sync.wait_ge(st[k], 16 * u)  # WAR: slot's previous store done
                if s >= 0:
                    sync.dma_start(
                        out=tiles[k][:, s:HW], in_=x[:, :, p, 0 : HW - s]
                    ).then_inc(ld[k], 16)
                else:
                    sync.dma_start(
                        out=tiles[k][:, 0 : HW + s], in_=x[:, :, p, -s:HW]
                    ).then_inc(ld[k], 16)

        @blk.vector
        def _(vector):
            for g in range(G):
                p = g % 8
                k = g % nslots
                u = g // nslots
                s = BASE[p]
                if s == 0:
                    continue
                vector.wait_ge(ld[k], 16 * (u + 1))
                rr = tiles[k][:].rearrange("p (h w) -> p h w", w=W)
                if s > 0:
                    vector.memset(rr[:, :, 0:s], 0.0).then_inc(ve[k], 1)
                else:
                    vector.memset(rr[:, :, W + s : W], 0.0).then_inc(ve[k], 1)

        @blk.scalar
        def _(scalar):
            ve_done = [0] * nslots
            st_done = [0] * nslots
            for g in range(G):
                p = g % 8
                k = g % nslots
                u = g // nslots
                s = BASE[p]
                if s == 0:
                    scalar.wait_ge(ld[k], 16 * (u + 1))
                else:
                    ve_done[k] += 1
                    scalar.wait_ge(ve[k], ve_done[k])
                scalar.dma_start(out=out[:, :, p, :], in_=tiles[k][:]).then_inc(
                    st[k], 16
                )
                st_done[k] += 1
            for k in range(nslots):
                scalar.wait_ge(st[k], 16 * st_done[k])

    return nc


def _build_aff(nc: bass.Bass, x, out, reps: int) -> bass.Bass:
    """Affine-stride scheme: the per-class shift s is affine in p within
    p in [0,5) (s = p-2) and p in [5,8) (s = 6-p), so one DMA per group can
    fold the shift into the p-stride of the SBUF-side access pattern.

    Group tile layout (per partition = one (b, c_hi)): class block p at
    base beta_p, holding the out-flat H*W content of that class. The load
    writes x[class p][j] to beta_p + s_p + j; choosing beta so that
    delta_p = beta_p + s_p is affine in p makes the load dst a single AP.
    Blocks are separated by small gaps that absorb the shift spill; DVE
    memsets zero the per-row edge columns afterward (same as v1).

    4 big DMAs total (2 loads + 2 stores), all 16 KiB contiguous runs.
    """
    from contextlib import ExitStack

    # group: (p0, n_classes, a, b) with s = a*p + b for p in [p0, p0+n)
    groups = [
        ("A", 0, 5, 1, -2),
        ("B", 5, 3, -1, 6),
    ]

    with ExitStack() as stack:
        tiles = {}
        for g, p0, n, a, b in groups:
            # load dst stride D = HW+4 (delta), store src stride HW+4-a*1?
            # delta stride = D; beta stride = D - a. Front guard needed when
            # the most-negative backward spill crosses beta_0: guard = max(0, -(s at p0)).
            D = HW + 4
            guard = max(0, -(a * p0 + b))
            free = guard + max(n * D, n * (D - a) + 4)
            tiles[g] = stack.enter_context(
                nc.sbuf_tensor(f"tile{g}", [128, free], mybir.dt.float32)
            )
        ld = {g[0]: stack.enter_context(nc.semaphore(f"ld{g[0]}")) for g in groups}
        ve = {g[0]: stack.enter_context(nc.semaphore(f"ve{g[0]}")) for g in groups}
        st = {g[0]: stack.enter_context(nc.semaphore(f"st{g[0]}")) for g in groups}
        blk = stack.enter_context(nc.Block())

        def load_dst(g, p0, n, a, b):
            D = HW + 4
            guard = max(0, -(a * p0 + b))
            t = tiles[g]
            # delta_0 = beta_0 + s(p0) = guard + s(p0) ... with beta_0 = guard
            d0 = guard + (a * p0 + b)
            return t[:, d0 : d0 + n * D].rearrange("p (q f) -> p q f", f=D)[:, :, 0:HW]

        def store_src(g, p0, n, a, b):
            D = HW + 4
            guard = max(0, -(a * p0 + b))
            bstride = D - a
            t = tiles[g]
            return t[:, guard : guard + n * bstride].rearrange(
                "p (q f) -> p q f", f=bstride
            )[:, :, 0:HW]

        def beta(g, p0, n, a, b, q):
            D = HW + 4
            guard = max(0, -(a * p0 + b))
            return guard + q * (D - a)

        n_memset = {
            g: sum(1 for q in range(n) if a * (p0 + q) + b != 0)
            for g, p0, n, a, b in groups
        }

        @blk.sync
        def _(sync):
            for r in range(reps):
                for g, p0, n, a, b in groups:
                    if r > 0:
                        sync.wait_ge(st[g], 16 * r)
                    sync.dma_start(
                        out=load_dst(g, p0, n, a, b), in_=x[:, :, p0 : p0 + n, :]
                    ).then_inc(ld[g], 16)

        @blk.vector
        def _(vector):
            for r in range(reps):
                for g, p0, n, a, b in groups:
                    vector.wait_ge(ld[g], 16 * (r + 1))
                    for q in range(n):
                        s = a * (p0 + q) + b
                        if s == 0:
                            continue
                        off = beta(g, p0, n, a, b, q)
                        rr = tiles[g][:, off : off + HW].rearrange(
                            "p (h w) -> p h w", w=W
                        )
                        if s > 0:
                            vector.memset(rr[:, :, 0:s], 0.0).then_inc(ve[g], 1)
                        else:
                            vector.memset(rr[:, :, W + s : W], 0.0).then_inc(ve[g], 1)

        @blk.scalar
        def _(scalar):
            for r in range(reps):
                for g, p0, n, a, b in groups:
                    scalar.wait_ge(ve[g], n_memset[g] * (r + 1))
                    scalar.dma_start(
                        out=out[:, :, p0 : p0 + n, :], in_=store_src(g, p0, n, a, b)
                    ).then_inc(st[g], 16)
            for g, p0, n, a, b in groups:
                scalar.wait_ge(st[g], 16 * reps)

    return nc


def _get_nc() -> bass.Bass:
    global _cached_nc
    if _cached_nc is None:
        _cached_nc = _build(reps=1, variant="v3")
    return _cached_nc


def _run(x: np.ndarray, **kwargs):
    """Shard, run on 8 cores, gather. Returns (out, BassKernelResults)."""
    x = np.ascontiguousarray(np.asarray(x, dtype=np.float32))
    assert x.shape == (B, C, H, W), x.shape
    shards = x.reshape(N_CORES, B_SH, C_HI, 8, HW)
    in_maps = [{"x": shards[i]} for i in range(N_CORES)]
    res = run_bass_kernel_spmd(_get_nc(), in_maps, core_ids=list(range(N_CORES)), **kwargs)
    out = np.concatenate(
        [np.asarray(res.results[i]["out"]).reshape(B_SH, C, H, W) for i in range(N_CORES)],
        axis=0,
    )
    return out, res


def kernel(x: np.ndarray) -> np.ndarray:
    # Retry once on transient device errors (e.g. a wedged NeuronCore left
    # over from a previous run); a fresh attempt typically recovers.
    try:
        out, _ = _run(x)
    except Exception:
        import time as _time

        _time.sleep(5)
        out, _ = _run(x)
    return out


# revision 27
# speedup vs baseline: 1.0232x; 1.0232x over previous
"""CycleFC per-channel W-shift kernel for 8 TRN2 NeuronCores.

Problem: x [32, 256, 64, 64] f32. out[b,c,h,w] = x[b,c,h,w-s] when
0 <= w-s < 64 else 0, with s = BASE[c % 8], BASE = [-2,-1,0,1,2,1,0,-1].

Sharding: data-parallel on batch, 4 batches per core, no communication.

Per-core scheme (pure data movement, HBM-roofline bound; submitted
variant "v3" = _build_slots_h2, 16 pipeline units of 1 MiB):
  view x as [4, 32, 8, H*W]  (b, c_hi, c%8, flat spatial)
  for each channel class p (shift s) and H-half:
    - DMA-load the flat block shifted by s elements into an SBUF tile
      [128 part = (b, c_hi), 2048] -> ~8 KiB contiguous runs (HWDGE/sync)
    - DVE-memset the per-row edge columns (w < s or w >= W+s) to zero
    - DMA-store the tile back fully aligned (HWDGE/scalar)
  Loads and stores ride separate HWDGE rings; per-unit semaphores chain
  load -> memset -> store; units pipeline freely against each other.
  Measured 92-124 us/pass across sessions (~94 us HBM roofline).

Other builder variants in this file (v1/v2/pair/split2/aff/ph/d2d/...)
are the experiments that selected v3; kernel() uses v3 only.
"""

import numpy as np

import concourse.bass as bass
import concourse.mybir as mybir
from concourse.bass_utils import run_bass_kernel_spmd

B, C, H, W = 32, 256, 64, 64
HW = H * W  # 4096
N_CORES = 8
B_SH = B // N_CORES  # 4
C_HI = C // 8  # 32
BASE = [-2, -1, 0, 1, 2, 1, 0, -1]  # shift per (c % 8)

_cached_nc = None


def _build(reps: int = 1, variant: str = "v1") -> bass.Bass:
    """variant:
    v1      - one load/memset/store unit per channel class (8 units)
    pair    - classes with equal shift share one unit (5 units)
    split2  - each class split into 2 DMAs along batch (8 units, 2 DMAs each)
    noshift - v1 with all shifts forced 0 (WRONG output; alignment probe)
    """
    from contextlib import ExitStack

    nc = bass.Bass()
    x = nc.declare_dram_parameter(
        "x", [B_SH, C_HI, 8, HW], mybir.dt.float32, isOutput=False
    )
    out = nc.declare_dram_parameter(
        "out", [B_SH, C_HI, 8, HW], mybir.dt.float32, isOutput=True
    )

    if variant == "aff":
        return _build_aff(nc, x, out, reps)
    if variant.startswith("v2"):
        nslots = int(variant[2:]) if len(variant) > 2 else 12
        return _build_slots(nc, x, out, reps, nslots)
    if variant == "ph":
        return _build_phased(nc, x, out, reps)
    if variant in ("ldonly", "d2draw", "d2d"):
        return _build_d2d(nc, x, out, reps, variant)
    if variant == "ldwide":
        return _build_ldwide(nc, x, out, reps)
    if variant.startswith("v3"):
        rest = variant[2:]
        gp_store = rest.startswith("g")
        if gp_store:
            rest = rest[1:]
        nslots = int(rest) if rest else 20
        return _build_slots_h2(nc, x, out, reps, nslots, split=2, gp_store=gp_store)
    if variant.startswith("v4"):
        nslots = int(variant[2:]) if len(variant) > 2 else 32
        return _build_slots_h2(nc, x, out, reps, nslots, split=4)

    # units: (name, class-tuple, shift)
    if variant == "pair":
        units = [
            ((0,), -2),
            ((1, 7), -1),
            ((2, 6), 0),
            ((3, 5), 1),
            ((4,), 2),
        ]
    elif variant == "noshift":
        units = [((p,), 0) for p in range(8)]
    else:  # v1, split2
        units = [((p,), BASE[p]) for p in range(8)]

    n_dma = 2 if variant == "split2" else 1  # DMAs per load/store unit
    U = len(units)

    def src_ap(ps, lo, hi):
        """x[:, :, ps, lo:hi] as one AP (ps is a stride-regular tuple)."""
        if len(ps) == 1:
            return x[:, :, ps[0], lo:hi]
        step = ps[1] - ps[0]
        return x[:, :, ps[0] : ps[1] + 1 : step, lo:hi]

    def dst_ap(ps):
        if len(ps) == 1:
            return out[:, :, ps[0], :]
        step = ps[1] - ps[0]
        return out[:, :, ps[0] : ps[1] + 1 : step, :]

    with ExitStack() as stack:
        tiles = [
            stack.enter_context(
                nc.sbuf_tensor(f"tile{u}", [128, len(ps) * HW], mybir.dt.float32)
            )
            for u, (ps, _) in enumerate(units)
        ]
        ld = [stack.enter_context(nc.semaphore(f"ld{u}")) for u in range(U)]
        ve = [stack.enter_context(nc.semaphore(f"ve{u}")) for u in range(U)]
        st = [stack.enter_context(nc.semaphore(f"st{u}")) for u in range(U)]
        blk = stack.enter_context(nc.Block())

        @blk.sync
        def _(sync):
            for r in range(reps):
                for u, (ps, s) in enumerate(units):
                    if r > 0:
                        sync.wait_ge(st[u], 16 * n_dma * r)  # WAR: prev store done
                    lo, hi = max(0, -s), HW + min(0, -s)
                    tl, th = max(0, s), HW + min(0, s)
                    t3 = tiles[u][:].rearrange("p (q f) -> p q f", f=HW)
                    if n_dma == 1:
                        sync.dma_start(
                            out=t3[:, :, tl:th], in_=src_ap(ps, lo, hi)
                        ).then_inc(ld[u], 16)
                    else:
                        half = 64  # partitions per half (= 2 of 4 batches)
                        sync.dma_start(
                            out=t3[0:half, :, tl:th],
                            in_=src_ap(ps, lo, hi)[0 : B_SH // 2],
                        ).then_inc(ld[u], 16)
                        sync.dma_start(
                            out=t3[half:128, :, tl:th],
                            in_=src_ap(ps, lo, hi)[B_SH // 2 : B_SH],
                        ).then_inc(ld[u], 16)

        @blk.vector
        def _(vector):
            for r in range(reps):
                for u, (ps, s) in enumerate(units):
                    if s == 0:
                        continue
                    vector.wait_ge(ld[u], 16 * n_dma * (r + 1))
                    rr = tiles[u][:].rearrange("p (q h w) -> p q h w", h=H, w=W)
                    if s > 0:
                        vector.memset(rr[:, :, :, 0:s], 0.0).then_inc(ve[u], 1)
                    else:
                        vector.memset(rr[:, :, :, W + s : W], 0.0).then_inc(ve[u], 1)

        @blk.scalar
        def _(scalar):
            for r in range(reps):
                for u, (ps, s) in enumerate(units):
                    if s == 0:
                        scalar.wait_ge(ld[u], 16 * n_dma * (r + 1))
                    else:
                        scalar.wait_ge(ve[u], r + 1)
                    if n_dma == 1:
                        scalar.dma_start(out=dst_ap(ps), in_=tiles[u][:]).then_inc(
                            st[u], 16
                        )
                    else:
                        scalar.dma_start(
                            out=dst_ap(ps)[0 : B_SH // 2], in_=tiles[u][0:64]
                        ).then_inc(st[u], 16)
                        scalar.dma_start(
                            out=dst_ap(ps)[B_SH // 2 : B_SH], in_=tiles[u][64:128]
                        ).then_inc(st[u], 16)
            for u in range(U):
                scalar.wait_ge(st[u], 16 * n_dma * reps)

    return nc


def _build_slots_h2(
    nc: bass.Bass, x, out, reps: int, nslots: int, split: int = 2, gp_store: bool = False
) -> bass.Bass:
    """Like _build_slots but each class is split into `split` H-chunks:
    8*split units per pass. Finer pipeline granularity shortens the
    single-pass ramp (first store starts after ~1 MiB instead of ~2 MiB)
    and the tail.

    Unit (p, hh) covers out-flat positions [hh*HW2, (hh+1)*HW2) of class p,
    where HW2 = HW/split (a whole number of H rows, so the per-row edge
    memset pattern is unchanged). The load reads x-flat [hh*HW2 - s, ...)
    clipped to [0, HW). gp_store issues stores on the gpsimd (SWDGE) queue
    instead of the scalar HWDGE ring.
    """
    from contextlib import ExitStack

    HW2 = HW // split
    UPP = 8 * split  # units per pass
    G = reps * UPP
    nslots = min(nslots, G)

    with ExitStack() as stack:
        tiles = [
            stack.enter_context(
                nc.sbuf_tensor(f"slot{k}", [128, HW2], mybir.dt.float32)
            )
            for k in range(nslots)
        ]
        ld = [stack.enter_context(nc.semaphore(f"ld{k}")) for k in range(nslots)]
        ve = [stack.enter_context(nc.semaphore(f"ve{k}")) for k in range(nslots)]
        st = [stack.enter_context(nc.semaphore(f"st{k}")) for k in range(nslots)]
        blk = stack.enter_context(nc.Block())

        # s=0 classes (2 and 6) first and last: the first store needs no
        # memset hop after its load (shorter single-pass ramp), and the
        # final store's dependency chain skips the DVE as well.
        CLS = [2, 0, 1, 3, 4, 5, 7, 6]

        def unit(g):
            j = g % UPP
            p, hh = CLS[j % 8], j // 8
            return p, hh, g % nslots, g // nslots

        @blk.sync
        def _(sync):
            for g in range(G):
                p, hh, k, u = unit(g)
                s = BASE[p]
                # tile[j'] = x[hh*HW2 + j' - s] for valid; src range in x-flat:
                lo = max(0, hh * HW2 - s)
                hi = min(HW, (hh + 1) * HW2 - s)
                tl = lo - (hh * HW2 - s)  # dst offset within tile
                if u > 0:
                    sync.wait_ge(st[k], 16 * u)
                sync.dma_start(
                    out=tiles[k][:, tl : tl + (hi - lo)], in_=x[:, :, p, lo:hi]
                ).then_inc(ld[k], 16)

        @blk.vector
        def _(vector):
            for g in range(G):
                p, hh, k, u = unit(g)
                s = BASE[p]
                if s == 0:
                    continue
                vector.wait_ge(ld[k], 16 * (u + 1))
                rr = tiles[k][:].rearrange("p (h w) -> p h w", w=W)
                if s > 0:
                    vector.memset(rr[:, :, 0:s], 0.0).then_inc(ve[k], 1)
                else:
                    vector.memset(rr[:, :, W + s : W], 0.0).then_inc(ve[k], 1)

        def store_prog(eng):
            ve_done = [0] * nslots
            st_done = [0] * nslots
            for g in range(G):
                p, hh, k, u = unit(g)
                s = BASE[p]
                if s == 0:
                    eng.wait_ge(ld[k], 16 * (u + 1))
                else:
                    ve_done[k] += 1
                    eng.wait_ge(ve[k], ve_done[k])
                eng.dma_start(
                    out=out[:, :, p, hh * HW2 : (hh + 1) * HW2], in_=tiles[k][:]
                ).then_inc(st[k], 16)
                st_done[k] += 1
            for k in range(nslots):
                eng.wait_ge(st[k], 16 * st_done[k])

        if gp_store:

            @blk.gpsimd
            def _(gp):
                store_prog(gp)

        else:

            @blk.scalar
            def _(scalar):
                store_prog(scalar)

    return nc


def _build_ldwide(nc: bass.Bass, x, out, reps: int) -> bass.Bass:
    """Load-only control with 2 classes per tile: 4 DMAs/rep of [128, 2*HW]
    with 32 KiB contiguous runs -> half the descriptors of ldonly. WRONG
    output; isolates whether HWDGE descriptor generation rate binds.
    """
    from contextlib import ExitStack

    with ExitStack() as stack:
        tiles = [
            stack.enter_context(
                nc.sbuf_tensor(f"tile{q}", [128, 2 * HW], mybir.dt.float32)
            )
            for q in range(4)
        ]
        ld = [stack.enter_context(nc.semaphore(f"ld{q}")) for q in range(4)]
        blk = stack.enter_context(nc.Block())

        @blk.sync
        def _(sync):
            for r in range(reps):
                for q in range(4):
                    # classes 2q, 2q+1 are adjacent: x[:, :, 2q:2q+2, :] is
                    # one 32 KiB contiguous run per (b, c_hi)
                    sync.dma_start(
                        out=tiles[q][:], in_=x[:, :, 2 * q : 2 * q + 2, :]
                    ).then_inc(ld[q], 16)
            for q in range(4):
                sync.wait_ge(ld[q], 16 * reps)

    return nc


def _build_d2d(nc: bass.Bass, x, out, reps: int, kind: str) -> bass.Bass:
    """DRAM->DRAM family.

    ldonly: HBM->SBUF loads only (WRONG output; pure-read rate control)
    d2draw: 8 shifted DRAM->DRAM block copies, no edge fix (WRONG output)
    d2d:    d2draw + per-row edge zeros DMA'd from a zeroed SBUF tile
    """
    from contextlib import ExitStack

    with ExitStack() as stack:
        if kind == "ldonly":
            tiles = [
                stack.enter_context(
                    nc.sbuf_tensor(f"tile{p}", [128, HW], mybir.dt.float32)
                )
                for p in range(8)
            ]
            ld = [stack.enter_context(nc.semaphore(f"ld{p}")) for p in range(8)]
            blk = stack.enter_context(nc.Block())

            @blk.sync
            def _(sync):
                for r in range(reps):
                    for p in range(8):
                        sync.dma_start(out=tiles[p][:], in_=x[:, :, p, :]).then_inc(
                            ld[p], 16
                        )
                for p in range(8):
                    sync.wait_ge(ld[p], 16 * reps)

            return nc

        zt = stack.enter_context(nc.sbuf_tensor("zt", [128, 128], mybir.dt.float32))
        st = [stack.enter_context(nc.semaphore(f"st{p}")) for p in range(8)]
        ez = [stack.enter_context(nc.semaphore(f"ez{p}")) for p in range(8)]
        vz = stack.enter_context(nc.semaphore("vz"))
        blk = stack.enter_context(nc.Block())

        @blk.vector
        def _(vector):
            if kind == "d2d":
                vector.memset(zt[:], 0.0).then_inc(vz, 1)

        @blk.sync
        def _(sync):
            for r in range(reps):
                for p in range(8):
                    s = BASE[p]
                    lo, hi = max(0, -s), HW + min(0, -s)
                    tl, th = max(0, s), HW + min(0, s)
                    sync.dma_start(
                        out=out[:, :, p, tl:th], in_=x[:, :, p, lo:hi]
                    ).then_inc(st[p], 16)
            for p in range(8):
                sync.wait_ge(st[p], 16 * reps)

        if kind == "d2d":

            @blk.gpsimd
            def _(gp):
                gp.wait_ge(vz, 1)
                for r in range(reps):
                    for p in range(8):
                        s = BASE[p]
                        if s == 0:
                            continue
                        gp.wait_ge(st[p], 16 * (r + 1))
                        o4 = out[:, :, p, :].rearrange("b c (h w) -> b c h w", w=W)
                        if s > 0:
                            dst = o4[:, :, :, 0:s]
                        else:
                            dst = o4[:, :, :, W + s : W]
                        with nc.allow_non_contiguous_dma(
                            reason="per-row edge zeros: |s| elems per row"
                        ):
                            gp.dma_start(out=dst, in_=zt[:, 0 : H * abs(s)]).then_inc(
                                ez[p], 16
                            )
                nz = sum(1 for p in range(8) if BASE[p] != 0)
                for p in range(8):
                    if BASE[p] != 0:
                        gp.wait_ge(ez[p], 16 * reps)

    return nc


def _build_phased(nc: bass.Bass, x, out, reps: int) -> bass.Bass:
    """v1 structure, but the store phase is gated on ALL loads/memsets of the
    pass: HBM sees a pure-read phase then a pure-write phase, avoiding
    read/write bus-turnaround mixing penalties. Memsets overlap the tail of
    the load phase. HBM is the only binding resource, so phasing loses no
    overlap; it only removes R/W interleaving.
    """
    from contextlib import ExitStack

    with ExitStack() as stack:
        tiles = [
            stack.enter_context(nc.sbuf_tensor(f"tile{p}", [128, HW], mybir.dt.float32))
            for p in range(8)
        ]
        ld = [stack.enter_context(nc.semaphore(f"ld{p}")) for p in range(8)]
        ve = [stack.enter_context(nc.semaphore(f"ve{p}")) for p in range(8)]
        st = [stack.enter_context(nc.semaphore(f"st{p}")) for p in range(8)]
        blk = stack.enter_context(nc.Block())

        @blk.sync
        def _(sync):
            for r in range(reps):
                if r > 0:
                    for p in range(8):
                        sync.wait_ge(st[p], 16 * r)  # write phase r-1 drained
                for p in range(8):
                    s = BASE[p]
                    if s >= 0:
                        sync.dma_start(
                            out=tiles[p][:, s:HW], in_=x[:, :, p, 0 : HW - s]
                        ).then_inc(ld[p], 16)
                    else:
                        sync.dma_start(
                            out=tiles[p][:, 0 : HW + s], in_=x[:, :, p, -s:HW]
                        ).then_inc(ld[p], 16)

        @blk.vector
        def _(vector):
            for r in range(reps):
                for p in range(8):
                    s = BASE[p]
                    if s == 0:
                        continue
                    vector.wait_ge(ld[p], 16 * (r + 1))
                    rr = tiles[p][:].rearrange("p (h w) -> p h w", w=W)
                    if s > 0:
                        vector.memset(rr[:, :, 0:s], 0.0).then_inc(ve[p], 1)
                    else:
                        vector.memset(rr[:, :, W + s : W], 0.0).then_inc(ve[p], 1)

        @blk.scalar
        def _(scalar):
            for r in range(reps):
                # gate: whole read phase (incl. memsets) done before any store
                for p in range(8):
                    s = BASE[p]
                    if s == 0:
                        scalar.wait_ge(ld[p], 16 * (r + 1))
                    else:
                        scalar.wait_ge(ve[p], r + 1)
                for p in range(8):
                    scalar.dma_start(out=out[:, :, p, :], in_=tiles[p][:]).then_inc(
                        st[p], 16
                    )
            for p in range(8):
                scalar.wait_ge(st[p], 16 * reps)

    return nc


def _build_slots(nc: bass.Bass, x, out, reps: int, nslots: int) -> bass.Bass:
    """v1 structure with a rotating pool of tile buffers so that, across the
    benchmark rep loop, unit g's load only waits for the store of unit
    g-nslots — a deep pipeline window that removes the per-unit
    load->store->load serialization. With reps=1 (the graded single pass)
    only 8 slots are touched and this is identical to v1.
    """
    from contextlib import ExitStack

    G = reps * 8
    nslots = min(nslots, G)

    with ExitStack() as stack:
        tiles = [
            stack.enter_context(nc.sbuf_tensor(f"slot{k}", [128, HW], mybir.dt.float32))
            for k in range(nslots)
        ]
        ld = [stack.enter_context(nc.semaphore(f"ld{k}")) for k in range(nslots)]
        ve = [stack.enter_context(nc.semaphore(f"ve{k}")) for k in range(nslots)]
        st = [stack.enter_context(nc.semaphore(f"st{k}")) for k in range(nslots)]
        blk = stack.enter_context(nc.Block())

        @blk.sync
        def _(sync):
            for g in range(G):
                p = g % 8
                k = g % nslots
                u = g // nslots
                s = BASE[p]
                if u > 0:
                    sync.wait_ge(st[k], 16 * u)  # WAR: slot's previous store done
                if s >= 0:
                    sync.dma_start(
                        out=tiles[k][:, s:HW], in_=x[:, :, p, 0 : HW - s]
                    ).then_inc(ld[k], 16)
                else:
                    sync.dma_start(
                        out=tiles[k][:, 0 : HW + s], in_=x[:, :, p, -s:HW]
                    ).then_inc(ld[k], 16)

        @blk.vector
        def _(vector):
            for g in range(G):
                p = g % 8
                k = g % nslots
                u = g // nslots
                s = BASE[p]
                if s == 0:
                    continue
                vector.wait_ge(ld[k], 16 * (u + 1))
                rr = tiles[k][:].rearrange("p (h w) -> p h w", w=W)
                if s > 0:
                    vector.memset(rr[:, :, 0:s], 0.0).then_inc(ve[k], 1)
                else:
                    vector.memset(rr[:, :, W + s : W], 0.0).then_inc(ve[k], 1)

        @blk.scalar
        def _(scalar):
            ve_done = [0] * nslots
            st_done = [0] * nslots
            for g in range(G):
                p = g % 8
                k = g % nslots
                u = g // nslots
                s = BASE[p]
                if s == 0:
                    scalar.wait_ge(ld[k], 16 * (u + 1))
                else:
                    ve_done[k] += 1
                    scalar.wait_ge(ve[k], ve_done[k])
                scalar.dma_start(out=out[:, :, p, :], in_=tiles[k][:]).then_inc(
                    st[k], 16
                )
                st_done[k] += 1
            for k in range(nslots):
                scalar.wait_ge(st[k], 16 * st_done[k])

    return nc


def _build_aff(nc: bass.Bass, x, out, reps: int) -> bass.Bass:
    """Affine-stride scheme: the per-class shift s is affine in p within
    p in [0,5) (s = p-2) and p in [5,8) (s = 6-p), so one DMA per group can
    fold the shift into the p-stride of the SBUF-side access pattern.

    Group tile layout (per partition = one (b, c_hi)): class block p at
    base beta_p, holding the out-flat H*W content of that class. The load
    writes x[class p][j] to beta_p + s_p + j; choosing beta so that
    delta_p = beta_p + s_p is affine in p makes the load dst a single AP.
    Blocks are separated by small gaps that absorb the shift spill; DVE
    memsets zero the per-row edge columns afterward (same as v1).

    4 big DMAs total (2 loads + 2 stores), all 16 KiB contiguous runs.
    """
    from contextlib import ExitStack

    # group: (p0, n_classes, a, b) with s = a*p + b for p in [p0, p0+n)
    groups = [
        ("A", 0, 5, 1, -2),
        ("B", 5, 3, -1, 6),
    ]

    with ExitStack() as stack:
        tiles = {}
        for g, p0, n, a, b in groups:
            # load dst stride D = HW+4 (delta), store src stride HW+4-a*1?
            # delta stride = D; beta stride = D - a. Front guard needed when
            # the most-negative backward spill crosses beta_0: guard = max(0, -(s at p0)).
            D = HW + 4
            guard = max(0, -(a * p0 + b))
            free = guard + max(n * D, n * (D - a) + 4)
            tiles[g] = stack.enter_context(
                nc.sbuf_tensor(f"tile{g}", [128, free], mybir.dt.float32)
            )
        ld = {g[0]: stack.enter_context(nc.semaphore(f"ld{g[0]}")) for g in groups}
        ve = {g[0]: stack.enter_context(nc.semaphore(f"ve{g[0]}")) for g in groups}
        st = {g[0]: stack.enter_context(nc.semaphore(f"st{g[0]}")) for g in groups}
        blk = stack.enter_context(nc.Block())

        def load_dst(g, p0, n, a, b):
            D = HW + 4
            guard = max(0, -(a * p0 + b))
            t = tiles[g]
            # delta_0 = beta_0 + s(p0) = guard + s(p0) ... with beta_0 = guard
            d0 = guard + (a * p0 + b)
            return t[:, d0 : d0 + n * D].rearrange("p (q f) -> p q f", f=D)[:, :, 0:HW]

        def store_src(g, p0, n, a, b):
            D = HW + 4
            guard = max(0, -(a * p0 + b))
            bstride = D - a
            t = tiles[g]
            return t[:, guard : guard + n * bstride].rearrange(
                "p (q f) -> p q f", f=bstride
            )[:, :, 0:HW]

        def beta(g, p0, n, a, b, q):
            D = HW + 4
            guard = max(0, -(a * p0 + b))
            return guard + q * (D - a)

        n_memset = {
            g: sum(1 for q in range(n) if a * (p0 + q) + b != 0)
            for g, p0, n, a, b in groups
        }

        @blk.sync
        def _(sync):
            for r in range(reps):
                for g, p0, n, a, b in groups:
                    if r > 0:
                        sync.wait_ge(st[g], 16 * r)
                    sync.dma_start(
                        out=load_dst(g, p0, n, a, b), in_=x[:, :, p0 : p0 + n, :]
                    ).then_inc(ld[g], 16)

        @blk.vector
        def _(vector):
            for r in range(reps):
                for g, p0, n, a, b in groups:
                    vector.wait_ge(ld[g], 16 * (r + 1))
                    for q in range(n):
                        s = a * (p0 + q) + b
                        if s == 0:
                            continue
                        off = beta(g, p0, n, a, b, q)
                        rr = tiles[g][:, off : off + HW].rearrange(
                            "p (h w) -> p h w", w=W
                        )
                        if s > 0:
                            vector.memset(rr[:, :, 0:s], 0.0).then_inc(ve[g], 1)
                        else:
                            vector.memset(rr[:, :, W + s : W], 0.0).then_inc(ve[g], 1)

        @blk.scalar
        def _(scalar):
            for r in range(reps):
                for g, p0, n, a, b in groups:
                    scalar.wait_ge(ve[g], n_memset[g] * (r + 1))
                    scalar.dma_start(
                        out=out[:, :, p0 : p0 + n, :], in_=store_src(g, p0, n, a, b)
                    ).then_inc(st[g], 16)
            for g, p0, n, a, b in groups:
                scalar.wait_ge(st[g], 16 * reps)

    return nc


def _get_nc() -> bass.Bass:
    global _cached_nc
    if _cached_nc is None:
        _cached_nc = _build(reps=1, variant="v3")
    return _cached_nc


def _run(x: np.ndarray, **kwargs):
    """Shard, run on 8 cores, gather. Returns (out, BassKernelResults)."""
    x = np.ascontiguousarray(np.asarray(x, dtype=np.float32))
    assert x.shape == (B, C, H, W), x.shape
    shards = x.reshape(N_CORES, B_SH, C_HI, 8, HW)
    in_maps = [{"x": shards[i]} for i in range(N_CORES)]
    res = run_bass_kernel_spmd(_get_nc(), in_maps, core_ids=list(range(N_CORES)), **kwargs)
    out = np.concatenate(
        [np.asarray(res.results[i]["out"]).reshape(B_SH, C, H, W) for i in range(N_CORES)],
        axis=0,
    )
    return out, res


def kernel(x: np.ndarray) -> np.ndarray:
    # Retry once on transient device errors (e.g. a wedged NeuronCore left
    # over from a previous run); a fresh attempt typically recovers.
    try:
        out, _ = _run(x)
    except Exception:
        import time as _time

        _time.sleep(5)
        out, _ = _run(x)
    return out
